# revision 13
# baseline (speedup 1.0000x reference)
"""Trainium2 Bass kernel for a single nGPT-style attention head (v2).

Computation (see reference): fused QKV projection, RoPE over the full head
dim, L2-normalize q/k scaled by sqk, causal SDPA with scale sqrt(d_model).

Sharding: data-parallel over batch - 8 batch elements, one per NeuronCore.

v2 design vs the v1 baseline (94.9us):
  - Q/K projections in fp8 (DoubleRow, 2x contraction per pass).  Any
    uniform scale error is absorbed by the L2 normalization; direction
    noise from fp8 is ~0.04 rms which perturbs the (tiny, ~+-0.03)
    scores by ~1e-4 - negligible.  V stays bf16.
  - RoPE restructured: 32*sqk^2 scale folded into host cos/sin tables;
    rotate-half + combine done by two accumulating PE matmuls (idn,
    smat) over DVE-produced bf16 products, so all heavy DVE ops run in
    2x (bf16/SBUF) mode.
  - Norm path: ||q||^2 via one FD-512 ones-matmul per side (psum rows 0
    and 32 of one bank) -> ACT Ln -> ACT Exp(-0.5x+bias) = rsqrt, with
    the activation table pinned to natural_log_exp_and_others so Exp/Ln
    share ONE table load.  Replaces 32 FD-1 matmuls + 32 PE transposes
    + DVE Quake chains of v1.
  - PSUM->SBUF drains on GpSimd (idle engine), v-transposes on the DMA
    XBAR (dma_start_transpose), scores exp in [128,2x512] pairs (one
    ACT op per 2 off-diagonal strips), ae accumulated 1024 wide.
  - Output shipped bf16, upcast on host.
"""

import numpy as np
import ml_dtypes

import concourse.bass as bass
import concourse.tile as tile
from concourse import bacc, mybir
from concourse.bass import ts, ds
from concourse.bass_utils import run_bass_kernel_spmd

# Surface compile-hook exceptions (the PJRT bridge swallows tracebacks).
try:
    import traceback
    import libneuronxla as _lnx

    if not getattr(_lnx, "_err_wrapped", False):
        _orig_cc = _lnx.neuronx_cc

        def _cc_wrapper(*a, **kw):
            try:
                return _orig_cc(*a, **kw)
            except BaseException:
                traceback.print_exc()
                raise

        _lnx.neuronx_cc = _cc_wrapper
        _lnx._err_wrapped = True
except Exception:
    pass

# Pin the activation table set to natural_log_exp_and_others so the Exp
# (scores) and Ln (norm rsqrt) calls resolve to one table -> a single
# ACT_TABLE_LOAD instead of a ~2.7us reload per Exp<->Ln transition.
import concourse.bacc as _bacc_mod

if not getattr(_bacc_mod, "_act_tables_pinned", False):
    _orig_gat = _bacc_mod.get_activation_tables

    def _gat_pinned(arch):
        t = _orig_gat(arch)
        return {k: (v if k == "natural_log_exp_and_others" else type(v)())
                for k, v in t.items()}

    _bacc_mod.get_activation_tables = _gat_pinned
    _bacc_mod._act_tables_pinned = True

AFT = mybir.ActivationFunctionType
ALU = mybir.AluOpType
F32 = mybir.dt.float32
BF16 = mybir.dt.bfloat16
FP8 = mybir.dt.float8e4
DR = mybir.MatmulPerfMode.DoubleRow

B, T_FULL, C, D = 8, 2048, 1024, 128
DBG = False
ROPE_BASE = 10000.0
P = 128
TB = 512            # token block (one PSUM bank of f32)
NCO = C // P        # contraction chunks for the QKV projection
H = P // 2


def build_nc(T=T_FULL, num_devices=8):
    from contextlib import ExitStack
    NTB = T // TB
    NKT = T // P
    nc = bacc.Bacc("TRN2", target_bir_lowering=False, debug=False,
                   num_devices=num_devices)

    x8 = nc.dram_tensor("x8", [P, NTB, NCO, TB], FP8,
                        kind="ExternalInput").ap()
    xb = nc.dram_tensor("xb", [P, NTB, NCO, TB], BF16,
                        kind="ExternalInput").ap()
    w8 = nc.dram_tensor("w8", [P, 2, NCO, D], FP8, kind="ExternalInput").ap()
    wv = nc.dram_tensor("wv", [P, NCO, D], BF16, kind="ExternalInput").ap()
    # tab: 0=cos*f, 1=sinswz*f  (f = 32*sqk^2 folded; k uses the same
    # tables, corrected via the exp bias on the k-norm row)
    tab = nc.dram_tensor("tab", [P, 2, T], BF16, kind="ExternalInput").ap()
    # packed small constants: [idn | smat | tri] (bf16)
    cpk = nc.dram_tensor("cpk", [P, 3 * P + 1], BF16,
                         kind="ExternalInput").ap()
    # f32 per-partition columns: col0 = onb (ones), col1 = exp bias rows
    cpf = nc.dram_tensor("cpf", [P, 2], F32, kind="ExternalInput").ap()
    outT = nc.dram_tensor("outT", [D, T], BF16, kind="ExternalOutput").ap()
    dbg = (nc.dram_tensor("dbg", [P, 2, TB], BF16, kind="ExternalOutput").ap()
           if DBG else None)
    dbgv = (nc.dram_tensor("dbgv", [P, 8, P], BF16, kind="ExternalOutput").ap()
            if DBG else None)
    dbgn = (nc.dram_tensor("dbgn", [33, 2, TB], F32, kind="ExternalOutput").ap()
            if DBG else None)

    with tile.TileContext(nc) as tc:
        with ExitStack() as ctx:
            const = ctx.enter_context(tc.tile_pool(name="const", bufs=1))
            work = ctx.enter_context(tc.tile_pool(name="work", bufs=2))
            expool = ctx.enter_context(tc.tile_pool(name="expool", bufs=6))
            ps_qk = ctx.enter_context(
                tc.tile_pool(name="ps_qk", bufs=2, space="PSUM"))
            ps_sc = ctx.enter_context(
                tc.tile_pool(name="ps_sc", bufs=2, space="PSUM"))
            ps_o = ctx.enter_context(
                tc.tile_pool(name="ps_o", bufs=1, space="PSUM"))
            ps_nr = ctx.enter_context(
                tc.tile_pool(name="ps_nr", bufs=1, space="PSUM"))

            # critical-path DMAs on the SP queue; x-bf16 bulk on the ACT
            # queue; tables on the DVE queue.
            cpk_sb = const.tile([P, 3 * P + 1], BF16)
            nc.sync.dma_start(cpk_sb, cpk)
            cpf_sb = const.tile([P, 2], F32)
            nc.sync.dma_start(cpf_sb, cpf)

            # engine warm-up: trigger each engine's library load and the
            # scalar engine's table load immediately; dummy matmuls ramp
            # the PE clock while inputs stream.
            tin1 = work.tile([P, 8], F32, tag="tin1")
            nc.vector.memset(tin1, 1.0)
            tin2 = work.tile([P, 8], F32, tag="tin2")
            nc.gpsimd.memset(tin2, 1.0)
            tin3 = work.tile([P, 8], F32, tag="tin3")
            nc.scalar.activation(tin3, tin1, AFT.Exp)

            idn_sb = cpk_sb[:, 0:P]
            smat_sb = cpk_sb[:, P:2 * P]
            tri_sb = cpk_sb[:, 2 * P:3 * P]
            onb_sb = cpk_sb[:, 3 * P:3 * P + 1]
            bias_sb = cpf_sb[:, 1:2]

            w8_sb = const.tile([P, 2, NCO, D], FP8)
            nc.sync.dma_start(w8_sb, w8)
            x8t = const.tile([P, NTB, NCO, TB], FP8)
            nc.sync.dma_start(x8t[:, 0], x8[:, 0])
            tabt = const.tile([P, 2, T], BF16)
            nc.sync.dma_start(tabt[:, :, ts(0, TB)], tab[:, :, ts(0, TB)])
            xbt = const.tile([P, NTB, NCO, TB], BF16)
            nc.scalar.dma_start(xbt[:, 0], xb[:, 0])
            wv_sb = const.tile([P, NCO, D], BF16)
            nc.scalar.dma_start(wv_sb, wv)
            nc.sync.dma_start(x8t[:, 1], x8[:, 1])
            nc.sync.dma_start(tabt[:, :, ts(1, TB)], tab[:, :, ts(1, TB)])

            for w in range(5):
                wsc = ps_sc.tile([P, 2, TB], F32, tag="sc", name=f"warm{w}")
                nc.tensor.matmul(wsc[:, 0, 0:3 * P + 1], idn_sb, cpk_sb,
                                 start=True, stop=True)

            qk = const.tile([P, 2, T], BF16)     # roped q^T | roped k^T
            vt = const.tile([P, NKT, P], BF16)   # v tiles [tk, e]

            # ---- pending attention work queue ----
            pend = []
            att_state = {}

            def drain(n):
                for _ in range(min(n, len(pend))):
                    pend.pop(0)()

            def make_att(J, part):
                """Queue attention work for query block J.

                part 0: off-diagonal strip PAIRS 0..2J-1 (prior blocks'
                k/v only) - drain during block J's own v-projection.
                part 1: the 4 diagonal strips + AV tail + finish."""
                q_blk = qk[:, 0, ts(J, TB)]
                nstr = 4 * (J + 1)
                if part == 0:
                    st = att_state[J] = {
                        "po": ps_o.tile([P, TB], F32, tag="po",
                                        name=f"po{J}"),
                        "ae": work.tile([P, 2, TB], BF16, tag="ae",
                                        name=f"ae{J}"),
                        "aep": (work.tile([P, 2, TB], BF16, tag="aep",
                                          name=f"aep{J}") if J > 0 else None),
                        "exs": {},
                    }
                else:
                    st = att_state[J]
                po, ae, aep, exs = st["po"], st["ae"], st["aep"], st["exs"]

                def emit_pair(ip):
                    # off-diagonal strips 2ip, 2ip+1: full width
                    sc = ps_sc.tile([P, 2, TB], F32, tag="sc",
                                    name=f"sc{J}p{ip}")
                    for s in (0, 1):
                        i = 2 * ip + s
                        nc.tensor.matmul(sc[:, s], qk[:, 1, ds(P * i, P)],
                                         q_blk, start=True, stop=True)
                    ex = expool.tile([P, 2, TB], BF16, tag="ex",
                                     name=f"ex{J}p{ip}")
                    nc.scalar.activation(ex, sc, AFT.Exp)
                    if ip == 0:
                        nc.gpsimd.tensor_copy(aep, ex)
                    else:
                        nc.gpsimd.tensor_add(aep, aep, ex)
                    exs[2 * ip] = (ex, 0, 0)
                    exs[2 * ip + 1] = (ex, 1, 0)

                def emit_diag(dr):
                    # diagonal strip 4J+dr, columns [128*dr:512) valid
                    i = 4 * J + dr
                    slot = dr % 2
                    if slot == 0:
                        st["dsc"] = ps_sc.tile([P, 2, TB], F32, tag="sc",
                                               name=f"sc{J}d{dr}")
                        st["dex"] = expool.tile([P, 2, TB], BF16, tag="ex",
                                                name=f"ex{J}d{dr}")
                    sc, ex = st["dsc"], st["dex"]
                    off = P * dr
                    w = TB - off
                    nc.tensor.matmul(sc[:, slot, ds(off, w)],
                                     qk[:, 1, ds(P * i, P)],
                                     q_blk[:, ds(off, w)],
                                     start=True, stop=True)
                    nc.scalar.activation(ex[:, slot, ds(off, w)],
                                         sc[:, slot, ds(off, w)], AFT.Exp)
                    nc.vector.tensor_mul(ex[:, slot, ds(off, P)],
                                         ex[:, slot, ds(off, P)], tri_sb)
                    if dr == 0:
                        nc.vector.memset(ae, 0.0)
                    nc.vector.tensor_add(ae[:, slot, ds(off, w)],
                                         ae[:, slot, ds(off, w)],
                                         ex[:, slot, ds(off, w)])
                    exs[i] = (ex, slot, off)

                def emit_av(i):
                    ex, slot, off = exs.pop(i)
                    w = TB - off
                    nc.tensor.matmul(po[:, ds(off, w)], vt[:, i],
                                     ex[:, slot, ds(off, w)],
                                     start=(i == 0), stop=(i == nstr - 1))

                def fin():
                    with nc.named_scope(f"fin{J}"):
                        red = ps_qk.tile([P, TB], F32, tag="qk",
                                         name=f"red{J}")
                        nc.tensor.matmul(red[0:1], onb_sb, ae[:, 0],
                                         start=True, stop=False)
                        halves = ([aep[:, 0], aep[:, 1]] if J > 0 else [])
                        for h in halves:
                            nc.tensor.matmul(red[0:1], onb_sb, h,
                                             start=False, stop=False)
                        nc.tensor.matmul(red[0:1], onb_sb, ae[:, 1],
                                         start=False, stop=True)
                        invd = work.tile([1, TB], F32, tag="invd")
                        nc.vector.reciprocal_approx_fast(out=invd,
                                                         in_=red[0:1])
                        bcd = work.tile([P, TB], F32, tag="bcd")
                        nc.gpsimd.partition_broadcast(bcd, invd)
                        ob = work.tile([P, TB], BF16, tag="ob")
                        nc.vector.tensor_mul(ob, po, bcd)
                        nc.sync.dma_start(outT[:, ts(J, TB)], ob)

                def pair_item(ip, J=J):
                    def run():
                        with nc.named_scope(f"att{J}p{ip}"):
                            emit_pair(ip)
                            if ip >= 1:
                                emit_av(2 * ip - 2)
                                emit_av(2 * ip - 1)
                    return run

                def diag_item(dr, J=J):
                    def run():
                        with nc.named_scope(f"att{J}d{dr}"):
                            emit_diag(dr)
                            i = 4 * J + dr
                            if i >= 2:
                                emit_av(i - 2)
                    return run

                def last():
                    with nc.named_scope(f"att{J}tail"):
                        emit_av(nstr - 2)
                        emit_av(nstr - 1)
                        fin()

                if part == 0:
                    for ip in range(2 * J):
                        pend.append(pair_item(ip))
                else:
                    for dr in range(4):
                        pend.append(diag_item(dr))
                    pend.append(last)

            def make_vblk(j):
                """Queue the v projection + v transpose of block j."""
                def vmm():
                    with nc.named_scope(f"vblk{j}"):
                        st = att_state[j]
                        ps = st["vps"] = ps_qk.tile(
                            [P, TB], F32, tag="qk", name=f"v{j}")
                        for co in range(NCO):
                            nc.tensor.matmul(
                                ps, wv_sb[:, co], xbt[:, j, co],
                                start=(co == 0), stop=(co == NCO - 1))

                def vtr():
                    with nc.named_scope(f"vtr{j}"):
                        vst = work.tile([P, TB], BF16, tag="vst")
                        nc.scalar.activation(vst, att_state[j]["vps"],
                                             AFT.Copy)
                        for c in range(4):
                            nc.sync.dma_start_transpose(
                                vt[:, 4 * j + c], vst[:, ts(c, P)])

                pend.append(vmm)
                pend.append(vtr)

            for j in range(NTB):
                # -------- q/k fp8 DoubleRow projections + drains ---------
                with nc.named_scope(f"qkv{j}"):
                    if j + 2 < NTB:
                        nc.sync.dma_start(x8t[:, j + 2], x8[:, j + 2])
                        nc.sync.dma_start(tabt[:, :, ts(j + 2, TB)],
                                          tab[:, :, ts(j + 2, TB)])
                    if j + 1 < NTB:
                        nc.scalar.dma_start(xbt[:, j + 1], xb[:, j + 1])
                    qd = []
                    rps = []
                    for g in range(2):
                        ps = ps_qk.tile([P, TB], F32, tag="qk",
                                        name=f"qk{j}g{g}")
                        for cp in range(NCO // 2):
                            nc.tensor.matmul(
                                ps, w8_sb[:, g, ds(2 * cp, 2)],
                                x8t[:, j, ds(2 * cp, 2)],
                                start=(cp == 0), stop=(cp == NCO // 2 - 1),
                                perf_mode=DR)
                        d = work.tile([P, TB], BF16, tag=f"qd{g}")
                        nc.vector.tensor_copy(d, ps)
                        qd.append(d)
                        drain(1)

                    # ---- squares, norm matmuls, rope products + matmuls --
                    nr = ps_nr.tile([P, TB], F32, tag="nr", name=f"nr{j}")
                    for g in range(2):
                        sq = work.tile([P, TB], BF16, tag=f"sq{g}")
                        nc.vector.tensor_mul(sq, qd[g], qd[g])
                        nc.tensor.matmul(nr[ds(32 * g, 1)], onb_sb, sq,
                                         start=True, stop=True)
                        m12 = work.tile([P, 2, TB], BF16, tag=f"m12{g}")
                        nc.vector.tensor_mul(m12[:, 0], qd[g],
                                             tabt[:, 0, ts(j, TB)])
                        nc.vector.tensor_mul(m12[:, 1], qd[g],
                                             tabt[:, 1, ts(j, TB)])
                        rp = ps_qk.tile([P, TB], F32, tag="qk",
                                        name=f"rp{j}g{g}")
                        nc.tensor.matmul(rp, idn_sb, m12[:, 0],
                                         start=True, stop=False)
                        nc.tensor.matmul(rp, smat_sb, m12[:, 1],
                                         start=False, stop=True)
                        rps.append(rp)
                        drain(1)

                # ---- rsqrt of norms via Ln/Exp on ACT + broadcasts -------
                with nc.named_scope(f"nrm{j}"):
                    ln33 = work.tile([33, TB], F32, tag="ln33")
                    nc.scalar.activation(ln33, nr[0:33], AFT.Ln)
                    invs = work.tile([33, TB], BF16, tag="invs")
                    nc.scalar.activation(invs, ln33, AFT.Exp,
                                         scale=-0.5, bias=bias_sb[0:33])
                    bcq = work.tile([P, TB], BF16, tag="bcq")
                    nc.gpsimd.partition_broadcast(bcq, invs[0:1])
                    invk0 = work.tile([1, TB], BF16, tag="invk0")
                    nc.sync.dma_start(invk0, invs[32:33])
                    bck = work.tile([P, TB], BF16, tag="bck")
                    nc.gpsimd.partition_broadcast(bck, invk0)
                    if DBG and j == 0:
                        nrc = work.tile([33, TB], F32, tag="nrc")
                        nc.vector.tensor_copy(nrc, nr[0:33])
                        nc.sync.dma_start(dbgn[:, 0], nrc)
                        ivc = work.tile([33, TB], F32, tag="ivc")
                        nc.vector.tensor_copy(ivc, invs)
                        nc.sync.dma_start(dbgn[:, 1], ivc)
                    drain(1)
                    nc.vector.tensor_mul(qk[:, 0, ts(j, TB)], rps[0], bcq)
                    drain(1)
                    nc.vector.tensor_mul(qk[:, 1, ts(j, TB)], rps[1], bck)
                    drain(1)

                # queue: off-diagonal pairs, the v block, then the
                # diagonal strips + finish.
                make_att(j, 0)
                make_vblk(j)
                make_att(j, 1)
            drain(len(pend))
            if DBG:
                nc.sync.dma_start(dbg[:, 0], qk[:, 0, ts(0, TB)])
                nc.sync.dma_start(dbg[:, 1], qk[:, 1, ts(0, TB)])
                nc.sync.dma_start(dbgv, vt[:, 0:8])

    nc.compile()
    return nc


def _host_tables(T, sqk):
    d = D
    inv_freq = 1.0 / (ROPE_BASE ** (np.arange(0, d, 2, dtype=np.float64) / d))
    t = np.arange(T, dtype=np.float64)
    freqs = np.outer(inv_freq, t)                 # [d/2, T]
    cosf = np.cos(np.concatenate([freqs, freqs], axis=0))   # [d, T]
    sinf = np.sin(np.concatenate([freqs, freqs], axis=0))
    # W2[c, t] = sign_{sigma(c)} * sin[sigma(c), t]  (sigma = half swap);
    # the smat matmul then yields out[d] = q[sigma(d)] * W2[sigma(d)]
    # = sign_d * q[sigma(d)] * sin[d] = rotate_half term.
    sign = np.concatenate([-np.ones(H), np.ones(H)])
    sigma = np.concatenate([np.arange(H) + H, np.arange(H)]).astype(int)
    W2 = sign[sigma, None] * sinf[sigma, :]
    f = (C ** 0.5) * np.asarray(sqk, np.float64) ** 2       # [d]
    tabh = np.stack([cosf * f[:, None], W2 * f[:, None]], axis=1)  # [d,2,T]

    a = np.arange(P)
    tri = (a[None, :] >= a[:, None]).astype(np.float64)  # [tk, tq]
    idn = np.eye(P)
    smat = np.zeros((P, P))
    smat[sigma, np.arange(P)] = 1.0
    cpkh = np.concatenate([idn, smat, tri, np.ones((P, 1))],
                          axis=1).astype(ml_dtypes.bfloat16)
    # f32 columns: ones | exp-bias (row 32 = -ln f, correcting the k side
    # for using the q-scaled tables; requires uniform sqk)
    cpfh = np.zeros((P, 2), np.float32)
    cpfh[:, 0] = 1.0
    cpfh[32, 1] = -np.log(f[0])
    return tabh.astype(ml_dtypes.bfloat16), cpkh, cpfh


TRACE = False
LAST_EXEC_NS = None
LAST_TRACE = None
LAST_INSTS = None


def _fp8(a):
    return np.clip(a, -240.0, 240.0).astype(ml_dtypes.float8_e4m3)


def kernel(x, W_qkv, sqk):
    global LAST_EXEC_NS, LAST_TRACE, LAST_INSTS
    x = np.asarray(x)
    W_qkv = np.asarray(W_qkv)
    sqk = np.asarray(sqk, np.float64)
    T = x.shape[1]
    NTB = T // TB
    assert np.allclose(sqk, sqk[0]), "kernel assumes uniform sqk"
    tabh, cpkh, cpfh = _host_tables(T, sqk)
    # w[g, p, co, d] = W_qkv[g*D + d, co*P + p]
    wt = np.ascontiguousarray(
        W_qkv.reshape(3, D, NCO, P).transpose(0, 3, 2, 1))
    w8h = _fp8(8.0 * wt[:2].transpose(1, 0, 2, 3))   # [P, 2, NCO, D]
    wvh = wt[2].astype(ml_dtypes.bfloat16)           # [P, NCO, D]
    in_maps = []
    for b in range(B):
        xt = np.ascontiguousarray(
            x[b].T.reshape(NCO, P, NTB, TB).transpose(1, 2, 0, 3))
        in_maps.append({
            "x8": _fp8(xt),
            "xb": xt.astype(ml_dtypes.bfloat16),
            "w8": w8h,
            "wv": wvh,
            "tab": tabh,
            "cpk": cpkh,
            "cpf": cpfh,
        })
    nc = build_nc(T=T, num_devices=B)
    res = run_bass_kernel_spmd(nc, in_maps, core_ids=list(range(B)),
                               trace=TRACE)
    LAST_EXEC_NS = res.exec_time_ns
    LAST_TRACE = (res.instructions_and_trace[1]
                  if res.instructions_and_trace else None)
    LAST_INSTS = (res.instructions_and_trace[0]
                  if res.instructions_and_trace else None)
    out = np.stack([r["outT"].T for r in res.results])  # [B, T, D]
    return np.ascontiguousarray(out).astype(np.float32)
